# revision 6
# baseline (speedup 1.0000x reference)
"""Soft-SLIC (segment_reduce) Trainium2 Bass kernel, 8-core SPMD.

Algorithm (reference soft_slic_all):
  features f [1, 20, 32768] fp32; S=512 superpixels; 10 iterations:
    t[n,s]   = -||f_n - c_s||^2 + ||f_n||^2 = 2 f_n.c_s - ||c_s||^2
    Q[n,s]   = softmax_s(t[n,s])       (per-point softmax over superpixels)
    num[c,s] = sum_n Q[n,s] f[c,n];  den[s] = sum_n Q[n,s]
    c        = num / (den + eps)
  outputs: (Q [1,S,N], labels=argmax_s Q [1,N], centers [1,C,S])

Sharding: points N split across 8 cores (4096 each). Each core computes its
t/Q tiles locally, produces partial [21,512] (num rows 0..19, den row 20),
which is AllReduced every iteration; the center update is replicated.

Device layout per core, per 128-point tile j (32 tiles):
  mm1:  t_psum[128,512] = FA[:, j*128:+128].T @ CA      (K=C+1 or split rows)
        FA rows: f channels (+ ones row); CA rows: 2c (+ -|c|^2 row)
        -> softmax needs NO row-max subtraction: t <= max||f_n||^2 ~ 38 (validated)
  ACT:  U = exp(t) fp32 -> SBUF, accum_out -> Z[128,1]
  DVE:  rinv = 1/Z;  fTs[128,21] = featT[:,j,:] * rinv   (folds softmax
        normalization into mm2's stationary operand instead of scaling U)
  mm2:  num_psum[21,512] += fTs.T @ U                     (accumulate over j)
Final iteration also: Qout = U*rinv -> DMA out; labels via max/max_index.

Host side: shard/prep inputs, transpose-gather per-core [N_loc,S] Q shards
into the full [S,N] output, concat labels, pick centers from core 0.
"""

import numpy as np

N_CORES = 8
C = 20
N_FULL = 32768
S = 512
N_LOC = N_FULL // N_CORES   # 4096
NT = N_LOC // 128           # 32 tiles of 128 points
N_ITER = 10
EPS = 1e-16

# mm1 operand mode: "fp32" (exact, 4 cyc/row) or "split" (bf16 3-term, 1 cyc/row)
MM1_MODE = "fp32"
# mm2 moving operand (U) dtype: "fp32" (4 cyc/row) or "fp32r" (1 cyc/row, reduced
# precision on HW - must be validated before use)
MM2_MODE = "fp32"

_CACHE = {}


def _build_module():
    import concourse.bacc as bacc
    import concourse.tile as tile
    import concourse.mybir as mybir
    from concourse import bass
    from contextlib import ExitStack

    fp32 = mybir.dt.float32
    bf16 = mybir.dt.bfloat16
    u32 = mybir.dt.uint32
    AF = mybir.ActivationFunctionType

    # FA/CA row count for mm1. Partition starts of compute APs must be
    # 32-aligned, so the ones/-cn row lives at partition 32 (rows 20..31 zero).
    if MM1_MODE == "fp32":
        KA = 33             # rows: f (0..19), zeros (20..31), ones (32)
        fa_dt = fp32
    else:
        KA = 98             # quads: f_hi, f_hi, f_lo, ones@96, ones@97
        fa_dt = bf16

    nc = bacc.Bacc(
        "TRN2", target_bir_lowering=False, debug=False,
        enable_asserts=False, num_devices=N_CORES,
    )

    feat = nc.dram_tensor("feat", [KA, N_LOC], fa_dt, kind="ExternalInput")
    KN = 33  # fTs/num width: f rows 0..19, zeros, den(ones) row 32
    RD = 32  # den row index
    featT = nc.dram_tensor("featT", [128, NT, KN], fp32, kind="ExternalInput")
    ca0 = nc.dram_tensor("ca0", [KA, S], fa_dt, kind="ExternalInput")
    q_out = nc.dram_tensor("q_out", [NT, 128, S], fp32, kind="ExternalOutput")
    lab_out = nc.dram_tensor("lab_out", [128, NT, 8], u32, kind="ExternalOutput")
    cen_out = nc.dram_tensor("cen_out", [C, S], fp32, kind="ExternalOutput")

    q_ap = q_out.ap()
    lab_ap = lab_out.ap()
    cen_ap = cen_out.ap()
    rg = [list(range(N_CORES))]

    with ExitStack() as ctx:
        tc = ctx.enter_context(tile.TileContext(nc))
        const = ctx.enter_context(tc.tile_pool(name="const", bufs=1))
        cap = ctx.enter_context(tc.tile_pool(name="cap", bufs=2))
        work = ctx.enter_context(tc.tile_pool(name="work", bufs=6))
        small = ctx.enter_context(tc.tile_pool(name="small", bufs=8))
        pst = ctx.enter_context(tc.tile_pool(name="pst", bufs=3, space="PSUM"))
        psn = ctx.enter_context(tc.tile_pool(name="psn", bufs=2, space="PSUM"))
        psm = ctx.enter_context(tc.tile_pool(name="psm", bufs=1, space="PSUM"))
        dram = ctx.enter_context(tc.tile_pool(name="dram", bufs=4, space="DRAM"))

        # --- constants / persistent ---
        feat_sb = const.tile([KA, N_LOC], fa_dt)
        nc.sync.dma_start(feat_sb[:], feat.ap()[:])
        featT_sb = const.tile([128, NT, KN], fp32)
        nc.sync.dma_start(featT_sb[:], featT.ap()[:])
        ones_1x20 = const.tile([1, C], fp32)
        nc.vector.memset(ones_1x20[:], 1.0)
        ones_20x1 = const.tile([C, 1], fp32)
        nc.vector.memset(ones_20x1[:], 1.0)
        eps_sb = const.tile([1, 1], fp32)
        nc.vector.memset(eps_sb[:], EPS)
        lab_sb = const.tile([128, NT, 8], u32)

        CA = cap.tile([KA, S], fa_dt, tag="ca")
        nc.sync.dma_start(CA[:], ca0.ap()[:])

        for it in range(N_ITER):
            last = it == N_ITER - 1
            Zbuf = small.tile([128, NT], fp32, tag="zbuf", bufs=2)
            num_ps = psn.tile([KN, S], fp32, tag="num")
            for j in range(NT):
                t_ps = pst.tile([128, S], fp32, tag="t")
                nc.tensor.matmul(
                    t_ps[:], feat_sb[:, bass.ts(j, 128)], CA[:],
                    start=True, stop=True,
                )
                U = work.tile([128, S], fp32, tag="u")
                nc.scalar.activation(
                    U[:], t_ps[:], AF.Exp, accum_out=Zbuf[:, j:j + 1],
                )
                rv = small.tile([128, 1], fp32, tag="rv", bufs=6)
                nc.vector.reciprocal(rv[:], Zbuf[:, j:j + 1])
                fTs = small.tile([128, KN], fp32, tag="fts", bufs=6)
                nc.vector.tensor_scalar_mul(fTs[:], featT_sb[:, j, :], rv[:])
                nc.tensor.matmul(
                    num_ps[:], fTs[:], U[:],
                    start=(j == 0), stop=(j == NT - 1),
                )
                if last:
                    qo = work.tile([128, S], fp32, tag="qo")
                    nc.vector.tensor_scalar_mul(qo[:], U[:], rv[:])
                    nc.sync.dma_start(q_ap[j], qo[:])
                    m8 = small.tile([128, 8], fp32, tag="m8", bufs=4)
                    nc.vector.max(m8[:], U[:])
                    nc.vector.max_index(lab_sb[:, j, :], m8[:], U[:])

            # cross-core reduction of [num; den]
            R = small.tile([KN, S], fp32, tag="r", bufs=2)
            nc.vector.tensor_copy(R[:], num_ps[:])
            ar_in = dram.tile([KN, S], fp32, tag="ari")
            ar_out = dram.tile([KN, S], fp32, tag="aro")
            nc.sync.dma_start(ar_in[:], R[:])
            nc.gpsimd.collective_compute(
                "AllReduce", mybir.AluOpType.add,
                replica_groups=rg, ins=[ar_in.opt()], outs=[ar_out.opt()],
            )
            G = small.tile([KN, S], fp32, tag="g", bufs=2)
            nc.sync.dma_start(G[:], ar_out[:])

            # replicated center update: c = num / (den + eps)
            dep = small.tile([1, S], fp32, tag="dep", bufs=2)
            nc.scalar.activation(dep[:], G[RD:RD + 1, :], AF.Identity,
                                 bias=eps_sb[:])
            rden = small.tile([1, S], fp32, tag="rden", bufs=2)
            nc.vector.reciprocal(rden[:], dep[:])
            rdb_ps = psm.tile([C, S], fp32, tag="rdb")
            nc.tensor.matmul(rdb_ps[:], ones_1x20[:], rden[:],
                             start=True, stop=True)
            cenc = small.tile([C, S], fp32, tag="cenc", bufs=2)
            nc.vector.tensor_mul(cenc[:], G[0:C, :], rdb_ps[:])
            if last:
                nc.sync.dma_start(cen_ap[:], cenc[:])
            else:
                sq = small.tile([C, S], fp32, tag="sq", bufs=2)
                nc.vector.tensor_mul(sq[:], cenc[:], cenc[:])
                cn_ps = psm.tile([1, S], fp32, tag="cn")
                nc.tensor.matmul(cn_ps[:], ones_20x1[:], sq[:],
                                 start=True, stop=True)
                CA2 = cap.tile([KA, S], fa_dt, tag="ca")
                nc.gpsimd.memset(CA2[:], 0.0)
                if MM1_MODE == "fp32":
                    nc.scalar.mul(CA2[0:C, :], cenc[:], 2.0)
                    nc.scalar.mul(CA2[RD:RD + 1, :], cn_ps[:], -1.0)
                else:
                    # rows: 2c_hi (0..19), 2c_lo (20..39), 2c_hi (40..59),
                    #       -cn_hi (60), -cn_lo (61)
                    c2 = small.tile([C, S], fp32, tag="c2", bufs=2)
                    nc.scalar.mul(c2[:], cenc[:], 2.0)
                    nc.scalar.copy(CA2[0:C, :], c2[:])           # bf16 round
                    lo = small.tile([C, S], fp32, tag="lo", bufs=2)
                    nc.vector.tensor_sub(lo[:], c2[:], CA2[0:C, :])
                    nc.scalar.copy(CA2[C:2 * C, :], lo[:])
                    nc.vector.tensor_copy(CA2[2 * C:3 * C, :], CA2[0:C, :])
                    cnf = small.tile([1, S], fp32, tag="cnf", bufs=2)
                    nc.scalar.mul(cnf[:], cn_ps[:], -1.0)
                    nc.scalar.copy(CA2[3 * C:3 * C + 1, :], cnf[:])
                    lo1 = small.tile([1, S], fp32, tag="lo1", bufs=2)
                    nc.vector.tensor_sub(lo1[:], cnf[:], CA2[3 * C:3 * C + 1, :])
                    nc.scalar.copy(CA2[3 * C + 1:3 * C + 2, :], lo1[:])
                CA = CA2

        nc.sync.dma_start(lab_ap[:], lab_sb[:])

    nc.compile()
    return nc


def _prep_inputs(features):
    import ml_dtypes

    f = np.ascontiguousarray(np.asarray(features, dtype=np.float32)[0])  # [C, N]
    c0 = f[:, :S]
    cn0 = np.sum(c0 * c0, axis=0, dtype=np.float32)
    KN = 33

    in_maps = []
    for k in range(N_CORES):
        sh = f[:, k * N_LOC:(k + 1) * N_LOC]
        z12 = np.zeros((12, N_LOC), np.float32)
        z12s = np.zeros((12, S), np.float32)
        if MM1_MODE == "fp32":
            feat_k = np.concatenate(
                [sh, z12, np.ones((1, N_LOC), np.float32)], axis=0)
            ca_k = np.concatenate(
                [2.0 * c0, z12s, -cn0[None, :]], axis=0).astype(np.float32)
        else:
            bf = ml_dtypes.bfloat16
            f_hi = sh.astype(bf).astype(np.float32)
            f_lo = (sh - f_hi).astype(bf)
            ones = np.ones((1, N_LOC), bf)
            zb = np.zeros((12, N_LOC), bf)
            # quads: f_hi@0, f_hi@32, f_lo@64, ones@96, ones@97
            feat_k = np.concatenate(
                [f_hi.astype(bf), zb, f_hi.astype(bf), zb, f_lo, zb,
                 ones, ones], axis=0)
            c2 = 2.0 * c0
            c2_hi = c2.astype(bf).astype(np.float32)
            c2_lo = (c2 - c2_hi).astype(bf)
            cn_hi = cn0.astype(bf).astype(np.float32)
            cn_lo = (cn0 - cn_hi).astype(bf)
            zbs = np.zeros((12, S), bf)
            ca_k = np.concatenate(
                [c2_hi.astype(bf), zbs, c2_lo, zbs, c2_hi.astype(bf), zbs,
                 -cn_hi[None, :].astype(bf), -cn_lo[None, :]], axis=0)
        fT = np.concatenate(
            [sh.T, np.zeros((N_LOC, 12), np.float32),
             np.ones((N_LOC, 1), np.float32)], axis=1)
        featT_k = np.ascontiguousarray(
            fT.reshape(NT, 128, KN).transpose(1, 0, 2))
        in_maps.append({
            "feat": np.ascontiguousarray(feat_k),
            "featT": featT_k,
            "ca0": np.ascontiguousarray(ca_k),
        })
    return in_maps


def _run(features, trace=False):
    from concourse import bass_utils

    if "nc" not in _CACHE:
        _CACHE["nc"] = _build_module()
    nc = _CACHE["nc"]
    in_maps = _prep_inputs(features)
    res = bass_utils.run_bass_kernel_spmd(
        nc, in_maps, core_ids=list(range(N_CORES)), trace=trace,
    )
    return res


def _assemble(res):
    Q = np.empty((1, S, N_FULL), np.float32)
    labels = np.empty((1, N_FULL), np.int32)
    for k in range(N_CORES):
        r = res.results[k]
        q_loc = r["q_out"].reshape(N_LOC, S)            # [n_loc, s]
        Q[0, :, k * N_LOC:(k + 1) * N_LOC] = q_loc.T
        lab = r["lab_out"][:, :, 0].astype(np.int32)    # [128, NT]
        labels[0, k * N_LOC:(k + 1) * N_LOC] = lab.T.reshape(N_LOC)
    centers = res.results[0]["cen_out"][None, :, :].astype(np.float32)
    return Q, labels, centers


def kernel(features):
    res = _run(features, trace=False)
    return _assemble(res)
